# revision 2
# baseline (speedup 1.0000x reference)
"""AGNNConv on 8 TRN2 NeuronCores — pure-compute streaming design, v2.

This platform (axon/PJRT TRN2) has no usable data-dependent DMA: the
custom SWDGE gather/scatter ucode crashes the device and the generic
indirect DMA path is a ~66us/call software queue.  So the kernel is
built exclusively from streaming DMA + compute engines:

  - Host (pure layout, no numerics): assign nodes to 64-slot windows
    with a greedy in-degree balancer so every window carries ~E/1568
    edges (tpw=8 tiles instead of 9 for node-aligned windows — ~11%
    less DMA and compute).  Partition edges by dst window (196 windows
    per core), pad each window's edge list to tpw tiles of 128 edge
    slots, and materialize per-edge operand rows fs = feat[src],
    fd = feat[dst] in the exact SBUF layout the device consumes.
  - Device per batch of windows (engine assignment chosen against the
    cost model: gpsimd tensor ops run at 0.42 efficiency, so the bulk
    elementwise work goes to ACT (squares share one activation-table
    set with Ln/Exp — no table reloads) and DVE (bf16 2x mode)):
      ACT:  sq = fs^2, sq2 = fd^2      (Square)
      DVE:  prod = fs*fd, row-reduces ss_s, ss_d, cos
      ACT:  rsqrt chain ln/exp, p = exp(beta*cos_hat)
      POOL: payload = [p*fs | p] bf16  (single remaining gpsimd mul)
      PE:   scatter — per 128-edge tile a one-hot matrix
            A[e, m] = (slot(dst) == m) shipped from host as fp8, and
            A^T @ payload accumulates [64 nodes, 33] in PSUM across
            the window's tiles; two windows share one PSUM tile.
      out = msg / s on evacuation.
  - Pad edge slots have all-zero fs/fd/one-hot rows: they stay finite
    through the norm chain and contribute nothing to the scatter.
"""

import sys

if "/opt/trn_rl_repo" not in sys.path:
    sys.path.insert(0, "/opt/trn_rl_repo")

import numpy as np

# Problem constants (hardcoded per harness contract)
N_NODES = 100000
N_EDGES = 1600000
D = 32
NCORES = 8
WSZ = 64           # dst window size (one-hot width)
NW = 196           # windows per core
NWIN = NCORES * NW # 1568 global windows
TPW = 8            # tiles (128 edge slots) per window (balanced bins)
WB = 14            # windows per compute batch (196 = 14*14), even
PW = D + 1         # payload width


def build_graph(nw, tpw, wb, wsz=WSZ, d=D, repeat=1,
                skip_pe=False, skip_a=False, skip_norm=False, skip_cos=False,
                skip_pay=False):
    import concourse.bass as bass
    import concourse.tile as tile
    from concourse import bacc, mybir
    from contextlib import nullcontext

    f32 = mybir.dt.float32
    bf16 = mybir.dt.bfloat16
    fp8 = mybir.dt.float8e4
    X = mybir.AxisListType.X
    ADD = mybir.AluOpType.add
    SQ = mybir.ActivationFunctionType.Square

    assert nw % wb == 0 and wb % 2 == 0
    nb = nw // wb
    tb = wb * tpw  # tiles per batch
    npair = nw // 2

    nc = bacc.Bacc(None, target_bir_lowering=False, debug=False)
    fs_p = nc.declare_dram_parameter("fs", [128, nw, tpw, d], bf16, isOutput=False)
    fd_p = nc.declare_dram_parameter("fd", [128, nw, tpw, d], bf16, isOutput=False)
    ah_p = nc.declare_dram_parameter("ah", [128, nw, tpw, wsz], fp8, isOutput=False)
    beta_p = nc.declare_dram_parameter("beta", [1], f32, isOutput=False)
    out_p = nc.declare_dram_parameter("out", [nw * wsz, d], f32, isOutput=True)

    outR = out_p[:].rearrange("(j m) c -> m j c", m=128)

    with tile.TileContext(nc) as tc:
        with tc.tile_pool(name="singles", bufs=1) as singles:
            beta_sb = singles.tile([128, 1], f32)
            nc.sync.dma_start(out=beta_sb[:], in_=beta_p[:].to_broadcast([128, 1]))
            eps_sb = singles.tile([128, 1], f32)
            nc.vector.memset(eps_sb[:], 1e-24)
            obuf = singles.tile([128, npair, d], f32)
            if skip_pe:
                nc.vector.memset(obuf[:], 0.0)
            # persistent stand-ins for ablated inputs (zeroed once so
            # downstream reads stay legal under the repeat loop)
            A_pers = cn_pers = pay_pers = None
            if skip_a:
                A_pers = singles.tile([128, wb, tpw, wsz], fp8)
                nc.vector.memset(A_pers[:], 0.0)
            if skip_cos and skip_norm:
                cn_pers = singles.tile([128, tb, 1], f32)
                nc.vector.memset(cn_pers[:], 0.5)
            if skip_pay:
                pay_pers = singles.tile([128, tb, PW], bf16)
                nc.vector.memset(pay_pers[:], 0.0)

            with (
                tc.tile_pool(name="inp", bufs=3) as inp,
                tc.tile_pool(name="ap_", bufs=2) as ap_,
                tc.tile_pool(name="med", bufs=2) as med,
                tc.tile_pool(name="sml", bufs=4) as sml,
                tc.tile_pool(name="ps_", bufs=4, space="PSUM") as ps_,
                tc.For_i(0, repeat, 1) if repeat > 1 else nullcontext(),
            ):
                for b in range(nb):
                    ws = slice(b * wb, (b + 1) * wb)
                    fs_t = inp.tile([128, wb, tpw, d], bf16)
                    nc.sync.dma_start(out=fs_t[:], in_=fs_p[:, ws, :, :])
                    fd_t = inp.tile([128, wb, tpw, d], bf16)
                    nc.scalar.dma_start(out=fd_t[:], in_=fd_p[:, ws, :, :])
                    if not skip_a:
                        A_t = ap_.tile([128, wb, tpw, wsz], fp8)
                        nc.sync.dma_start(out=A_t[:], in_=ah_p[:, ws, :, :])
                        A_f = A_t[:].rearrange("i w t m -> i (w t) m")
                    else:
                        A_f = A_pers[:].rearrange("i w t m -> i (w t) m")

                    fsf = fs_t[:].rearrange("i w t c -> i (w t) c")
                    fdf = fd_t[:].rearrange("i w t c -> i (w t) c")

                    cn = sml.tile([128, tb, 1], f32)
                    if not skip_cos:
                        # cos numerator: DVE mul (bf16 2x) + DVE row-reduce
                        prod = med.tile([128, tb, d], bf16)
                        nc.vector.tensor_mul(prod[:], fsf, fdf)
                        cos = sml.tile([128, tb, 1], f32)
                        nc.vector.tensor_reduce(cos[:], prod[:], axis=X, op=ADD)
                    if not skip_norm:
                        # squared norms: ACT Square, DVE row-reduce
                        sq = med.tile([128, tb, d], bf16)
                        nc.scalar.activation(sq[:], fsf, SQ)
                        ss_s = sml.tile([128, tb, 1], f32)
                        nc.vector.tensor_reduce(ss_s[:], sq[:], axis=X, op=ADD)
                        sq2 = med.tile([128, tb, d], bf16)
                        nc.scalar.activation(sq2[:], fdf, SQ)
                        ss_d = sml.tile([128, tb, 1], f32)
                        nc.vector.tensor_reduce(ss_d[:], sq2[:], axis=X, op=ADD)

                        # rn = 1/sqrt(ss_s*ss_d + eps) = exp(-0.5*ln(.))
                        # (Ln/Exp/Square share one Act table set: no reloads)
                        ssp = sml.tile([128, tb, 1], f32)
                        nc.vector.tensor_mul(ssp[:], ss_s[:], ss_d[:])
                        lg = sml.tile([128, tb, 1], f32)
                        nc.scalar.activation(
                            lg[:], ssp[:], mybir.ActivationFunctionType.Ln,
                            bias=eps_sb[:],
                        )
                        rn = sml.tile([128, tb, 1], f32)
                        nc.scalar.activation(
                            rn[:], lg[:], mybir.ActivationFunctionType.Exp,
                            scale=-0.5,
                        )
                        if not skip_cos:
                            nc.vector.tensor_mul(cn[:], cos[:], rn[:])
                        else:
                            nc.vector.tensor_copy(out=cn[:], in_=rn[:])
                    elif not skip_cos:
                        nc.vector.tensor_copy(out=cn[:], in_=cos[:])
                    else:
                        cn = cn_pers

                    # p = exp(beta*cn)
                    p_t = sml.tile([128, tb, 1], bf16)
                    nc.scalar.activation(
                        p_t[:], cn[:], mybir.ActivationFunctionType.Exp,
                        scale=beta_sb[:],
                    )

                    # payload [p*fs | p] in bf16 (POOL's single big mul)
                    if not skip_pay:
                        pay = med.tile([128, tb, PW], bf16)
                        nc.gpsimd.tensor_mul(
                            pay[:, :, 0:d], fsf, p_t[:].to_broadcast([128, tb, d])
                        )
                        nc.vector.tensor_copy(out=pay[:, :, d : d + 1], in_=p_t[:])
                    else:
                        pay = pay_pers

                    # scatter: PSUM accumulation; 2 windows per PSUM tile
                    hb = wb // 2
                    if not skip_pe:
                        stg = med.tile([128, hb, PW], f32)
                        for pj in range(hb):
                            ps = ps_.tile([128, PW], f32)
                            for h in range(2):
                                wj = pj * 2 + h
                                for t in range(tpw):
                                    ti = wj * tpw + t
                                    nc.tensor.matmul(
                                        ps[h * wsz : (h + 1) * wsz, :],
                                        lhsT=A_f[:, ti, :],
                                        rhs=pay[:, ti, :],
                                        start=(t == 0),
                                        stop=(t == tpw - 1),
                                    )
                            nc.scalar.copy(out=stg[:, pj, :], in_=ps[:])
                        scb = sml.tile([128, hb, 1], f32)
                        nc.vector.tensor_scalar_max(
                            scb[:], stg[:, :, d : d + 1], 1e-30
                        )
                        rcb = sml.tile([128, hb, 1], f32)
                        nc.vector.reciprocal(rcb[:], scb[:])
                        nc.vector.tensor_mul(
                            obuf[:, b * hb : (b + 1) * hb, :],
                            stg[:, :, 0:d],
                            rcb[:].to_broadcast([128, hb, d]),
                        )

            nc.sync.dma_start(out=outR[:, :, :], in_=obuf[:])

    nc.compile()
    return nc


def _balance_windows(deg, nwin, wsz):
    """Greedy LPT bin packing: nodes (desc in-degree) into nwin windows of
    wsz node slots, balancing per-window edge counts.  Returns (win_of,
    slot_of) node->window assignments."""
    import heapq

    n = deg.size
    win_of = np.empty(n, np.int64)
    slot_of = np.empty(n, np.int64)
    fill = np.zeros(nwin, np.int64)
    order = np.argsort(-deg, kind="stable")
    heap = [(0, w) for w in range(nwin)]
    heapq.heapify(heap)
    deg_l = deg.tolist()
    for idx in order.tolist():
        while True:
            e, w = heapq.heappop(heap)
            if fill[w] < wsz:
                break
        win_of[idx] = w
        slot_of[idx] = fill[w]
        fill[w] += 1
        if fill[w] < wsz:
            heapq.heappush(heap, (e + deg_l[idx], w))
    return win_of, slot_of


def host_prep(feat, beta, src, dst, ncores=NCORES, nw=NW, d=D, wsz=WSZ):
    """Pure index/layout prep. Returns (per-core input maps, tpw, pos_of)."""
    import ml_dtypes

    feat = np.ascontiguousarray(np.asarray(feat, dtype=np.float32))
    beta = np.ascontiguousarray(np.asarray(beta, dtype=np.float32))
    src = np.asarray(src).astype(np.int64)
    dst = np.asarray(dst).astype(np.int64)
    nwin = ncores * nw
    n_nodes = feat.shape[0]

    deg = np.bincount(dst, minlength=n_nodes)
    win_of, slot_of = _balance_windows(deg, nwin, wsz)

    ewin = win_of[dst]
    order = np.argsort(ewin, kind="stable")
    src_s, dst_s, win_s = src[order], dst[order], ewin[order]
    wcnt = np.bincount(win_s, minlength=nwin)
    tpw = max(TPW, int(-(-int(wcnt.max()) // 128)))
    starts = np.concatenate([[0], np.cumsum(wcnt)[:-1]])
    rank = np.arange(src_s.size) - starts[win_s]
    t_all = rank // 128
    i_all = rank % 128

    feat_bf = feat.astype(ml_dtypes.bfloat16)
    eye = np.eye(wsz, dtype=ml_dtypes.float8_e4m3fn)
    slot_e = slot_of[dst_s]

    in_maps = []
    for c in range(ncores):
        lo_w, hi_w = c * nw, (c + 1) * nw
        sel = (win_s >= lo_w) & (win_s < hi_w)
        e_src, e_win, e_slot, t_, i_ = (
            src_s[sel], win_s[sel] - lo_w, slot_e[sel], t_all[sel], i_all[sel],
        )
        e_dst = dst_s[sel]

        fs = np.zeros((128, nw, tpw, d), dtype=ml_dtypes.bfloat16)
        fd = np.zeros((128, nw, tpw, d), dtype=ml_dtypes.bfloat16)
        ah = np.zeros((128, nw, tpw, wsz), dtype=ml_dtypes.float8_e4m3fn)
        fs[i_, e_win, t_] = feat_bf[e_src]
        fd[i_, e_win, t_] = feat_bf[e_dst]
        ah[i_, e_win, t_] = eye[e_slot]

        in_maps.append({"fs": fs, "fd": fd, "ah": ah, "beta": beta})

    pos_of = win_of * wsz + slot_of  # global output row of each node
    return in_maps, tpw, pos_of


_CACHED = {}


def kernel(feat, beta, src, dst):
    from concourse.bass_utils import run_bass_kernel_spmd

    in_maps, tpw, pos_of = host_prep(feat, beta, src, dst)
    key = ("nc", tpw)
    if key not in _CACHED:
        _CACHED[key] = build_graph(NW, tpw, WB)
    nc = _CACHED[key]
    res = run_bass_kernel_spmd(nc, in_maps, list(range(NCORES))).results
    full = np.concatenate([res[c]["out"] for c in range(NCORES)], axis=0)
    return full[pos_of].astype(np.float32)
